# revision 45
# baseline (speedup 1.0000x reference)
"""RNN-T joint network kernel for 8 Trainium2 NeuronCores.

out[b,t,u,:] = W2 @ tanh(W1e @ enc[b,t] + W1d @ dec[b,u] + b1) + b2

Shapes: B=4, T=200, U=100, D=512, H=1024, O=512.
Sharding: T split 8 ways (25 t's per core); dec + weights replicated.

All matmul inputs are bf16 (converted on host): halves input DMA, enables
FWL fast weight loads, and removes the fp32r cast pass.  Output is written
bf16 and upcast on host (norm rel err ~3.5e-3, well under the 2e-2 gate).
Phase-2 matmul streaming (32 MMs x rows per 500-row chunk = 133us/core at
1 col/cycle) is the hard floor; everything else is scheduled to hide under
the bandwidth-capped input load (~3.5MB/core, ~8.5-18us).

Per-core device program:
  Warmup: ~40 dummy matmuls keep the PE busy from the end of the engine
          preamble so the HAM clock-gate is at 8/8 (2.4 GHz) for phase 1.
  Input:  8 DMAs of ~0.5-0.7MB on the two HWDGE rings in strict need-order:
          encT, then w1 quarter-PAIRS (piece q = [w1e hk2q,2q+1 | w1d ...],
          with the bf16 biases and decT halves riding inside pieces 0/1 --
          tiny standalone DMAs poison a queue), then the two oc-major w2
          halves last.
  Phase 1: per hk, enc then dec matmuls (4 dk accumulating each), paced by
          the arriving w1 pieces; PSUM evacs (+b1) run on ACT into k-PAIR
          ench/dech tiles so the DVE FIFO holds only builds.
  Overlay: the first 3 chunks' s-tiles are built pair-by-pair inside the
          phase-1 loop (DVE broadcast-add + ACT tanh per pair), so when the
          first w2 half lands the PE goes straight to dense phase-2 flow.
  Phase 2: per chunk (up to 5 t's = 500 rows): DVE builds s[k-pair] =
          dec_hT (+) enc_hT bcast (stride-0 APs, fp32 in -> bf16 out), one
          in-place tanh over all k (ACT), then 4 oc x 8 k accumulating bf16
          matmuls against stationary W2 blocks -> psum out^T [128, rows];
          +b2 on the psum->sbuf copy (oc0,1 ACT / oc2,3 DVE) into one
          [128, 4*rows] staging tile; ONE output DMA per chunk, rings
          alternating.  Lead-in chunks are [2,3] t's, drain chunks [4,1].
"""

from contextlib import ExitStack

import numpy as np
import ml_dtypes

import concourse.bacc as bacc
import concourse.bass as bass
import concourse.mybir as mybir
import concourse.tile as tile
from concourse.bass_utils import run_bass_kernel_spmd

F32 = mybir.dt.float32
BF16 = mybir.dt.bfloat16
NPBF16 = ml_dtypes.bfloat16

B, T, U, D, H, O = 4, 200, 100, 512, 1024, 512
NCORES = 8
TLOC = T // NCORES            # 25 t's per core
PAIRS = B * TLOC              # 100 (b,t) pairs per core
TCH = 5                       # t's per inner chunk
CHROWS = TCH * U              # 500 rows per chunk
NCH = TLOC // TCH             # 5 chunks per b
ROWS = PAIRS * U              # 10000 output rows per core
DK = D // 128                 # 4 contraction chunks for phase 1
HK = H // 128                 # 8 h chunks

_CACHE = {}


def _build():
    nc = bacc.Bacc("TRN2", target_bir_lowering=False, debug=False,
                   num_devices=NCORES)
    # inputs arrive pre-interleaved in SBUF layout: [128, nchunk*width],
    # partition p holding chunk k's row (k*128+p) at cols [k*width, ...)
    encT = nc.dram_tensor("encT", [128, DK * PAIRS], BF16, kind="ExternalInput")
    # w1 is delivered as quarter-PAIRS: piece q = [w1e hk(2q,2q+1) | w1d
    # hk(2q,2q+1)], each 0.5MB -- big enough for good DMA throughput, small
    # enough that phase-1 paces along with the stream.  The bf16 biases ride
    # at the tail of piece 0: standalone 32B-descriptor DMAs poison a HWDGE
    # queue for microseconds.
    BIASC = HK + O // 128
    # decT halves ride inside pieces 0 and 1 so the dec data arrives with
    # its weights (no separate decT transfer to stall on)
    W1QC = 2 * HK * D + BIASC + DK * B * U
    w1qT = nc.dram_tensor("w1qT", [128, W1QC], BF16, kind="ExternalInput")
    # w2 layout is oc-major: [128, oc, hk, 128] -> the first matmul groups
    # (oc=0,1) only need the first half
    w2T = nc.dram_tensor("w2T", [128, HK * O], BF16, kind="ExternalInput")
    out = nc.dram_tensor("out", [O, ROWS], BF16, kind="ExternalOutput")

    BU = B * U
    QHK = 2                    # hk per w1 quarter
    with tile.TileContext(nc) as tc, ExitStack() as ctx:
        consts = ctx.enter_context(tc.tile_pool(name="consts", bufs=1))
        spool = ctx.enter_context(tc.tile_pool(name="spool", bufs=4))
        opool = ctx.enter_context(tc.tile_pool(name="opool", bufs=4))
        psB = ctx.enter_context(tc.tile_pool(name="psB", bufs=8, space="PSUM"))

        # ---- PE warmup: dummy matmuls so HAM un-throttles before phase 1 ----
        warm = consts.tile([128, 128], BF16, name="warm")
        nc.vector.memset(warm[:], 0.0)
        for _ in range(36):
            pw = psB.tile([128, 512], F32, tag="psB", name="pw")
            nc.tensor.matmul(pw[:, :128], lhsT=warm[:], rhs=warm[:],
                             start=True, stop=True)

        # ---- load inputs: both HWDGE rings, ~0.5MB pieces in strict
        # need-order, w2 halves last.  All 8 cores pull their copies
        # concurrently so the load is bandwidth-capped; phase 1 paces
        # along with the arriving quarter-pairs.
        QW = 2 * QHK * D
        L0 = QW + BIASC + 2 * BU      # piece0 + biases + dec dk0-1
        L1 = QW + 2 * BU              # piece1 + dec dk2-3
        w1q_s = [consts.tile([128, [L0, L1, QW, QW][i]], BF16,
                             name=f"w1q{i}") for i in range(4)]
        w2_s = [consts.tile([128, 2 * H], BF16, name=f"w2{i}")
                for i in range(2)]
        encT_s = consts.tile([128, DK * PAIRS], BF16)
        nc.sync.dma_start(encT_s[:], encT[:])
        nc.scalar.dma_start(w1q_s[0][:], w1qT[:, :L0])
        nc.sync.dma_start(w1q_s[1][:], w1qT[:, L0:L0 + L1])
        nc.scalar.dma_start(w1q_s[2][:], w1qT[:, L0 + L1:][:, :QW])
        nc.sync.dma_start(w1q_s[3][:], w1qT[:, L0 + L1 + QW:][:, :QW])
        nc.scalar.dma_start(w2_s[0][:], w2T[:, :2 * H])
        nc.sync.dma_start(w2_s[1][:], w2T[:, 2 * H:])
        # biases ride bf16 in the w1q0 piece; cast to fp32 once (DVE wants
        # fp32 scalars)
        b12f = consts.tile([128, BIASC], F32)
        nc.vector.tensor_copy(b12f[:], w1q_s[0][:, QW:QW + BIASC])
        b1_s = b12f[:, :HK]
        b2c_s = b12f[:, HK:]

        def dec_ap(dk):
            if dk < 2:
                return w1q_s[0][:, QW + BIASC + dk * BU:][:, :BU]
            return w1q_s[1][:, QW + (dk - 2) * BU:][:, :BU]

        def w1e_ap(hk, dk):
            return w1q_s[hk // QHK][:, (hk % QHK) * D + dk * 128:][:, :128]

        def w1d_ap(hk, dk):
            return w1q_s[hk // QHK][:, QHK * D + (hk % QHK) * D + dk * 128:][:, :128]

        def w2_ap(k, oc):
            return w2_s[oc // 2][:, (oc % 2) * H + k * 128:][:, :128]

        # ---- phase 1: enc_hT (+b1) and dec_hT, interleaved per hk ----
        # k-PAIR tiles (matching the w1 quarter DMAs): fine-grained enough
        # that phase-2 builds start as each pair is ready, and wide enough
        # that each build covers two k's in one DVE instruction.
        # evacs live on ACT so the DVE FIFO holds only builds.
        NP_ = HK // QHK
        ench_p = [consts.tile([128, QHK * PAIRS], F32, name=f"ench{q}")
                  for q in range(NP_)]
        dech_p = [consts.tile([128, QHK * BU], F32, name=f"dech{q}")
                  for q in range(NP_)]

        # phase-2 chunk table; the first OVERLAY chunks' builds+tanh are
        # emitted inside the phase-1 loop (pair-by-pair, as each ench/dech
        # pair lands) so all elementwise work for them is finished by the
        # time the w2 halves arrive and the PE can go dense immediately
        chunks = []
        for b in range(B):
            if b == 0:
                sizes = [2, 3] + [4] * 5
            elif b == B - 1:
                sizes = [TCH] * 4 + [4, 1]
            else:
                sizes = [TCH] * NCH
            t0c = 0
            for tch in sizes:
                chunks.append((b, t0c, tch))
                t0c += tch
        OVERLAY = 3
        ov_st = [spool.tile([128, HK * CHROWS], BF16, tag="s", name=f"sov{i}")
                 for i in range(OVERLAY)]

        def build_pair(s_t, b, t0c, tch, q, fuse_tanh):
            rows_c = tch * U
            c0 = b * TLOC + t0c
            in0 = dech_p[q][:].rearrange("p (k u) -> p k u", k=QHK)
            in0 = in0[:, :, b * U:(b + 1) * U].rearrange(
                "p k (a u) -> p k a u", a=1)
            in1 = ench_p[q][:].rearrange("p (k c) -> p k c", k=QHK)
            in1 = in1[:, :, c0:c0 + tch].rearrange(
                "p k (t a) -> p k t a", a=1)
            bc0, bc1 = bass.broadcast_tensor_aps(in0, in1)
            outap = s_t[:, q * QHK * CHROWS:(q + 1) * QHK * CHROWS]
            outap = outap.rearrange("p (k c) -> p k c", k=QHK)
            outap = outap[:, :, :rows_c].rearrange(
                "p k (t u) -> p k t u", t=tch)
            nc.vector.tensor_tensor(outap, bc0, bc1, mybir.AluOpType.add)
            if fuse_tanh:
                su = s_t[:, q * QHK * CHROWS:(q + 1) * QHK * CHROWS]
                su = su.rearrange("p (k c) -> p k c", k=QHK)[:, :, :rows_c]
                nc.scalar.activation(su, su,
                                     mybir.ActivationFunctionType.Tanh)

        for hk in range(HK):
            q, r = hk // QHK, hk % QHK
            pe = psB.tile([128, 512], F32, tag="psB", name="pe")
            pe = pe[:, :PAIRS]
            for dk in range(DK):
                nc.tensor.matmul(
                    pe[:],
                    lhsT=w1e_ap(hk, dk),
                    rhs=encT_s[:, dk * PAIRS:(dk + 1) * PAIRS],
                    start=(dk == 0), stop=(dk == DK - 1),
                )
            nc.scalar.activation(ench_p[q][:, r * PAIRS:(r + 1) * PAIRS], pe[:],
                                 mybir.ActivationFunctionType.Identity,
                                 bias=b1_s[:, hk:hk + 1])
            pd = psB.tile([128, 512], F32, tag="psB", name="pd")
            pd = pd[:, :BU]
            for dk in range(DK):
                nc.tensor.matmul(
                    pd[:],
                    lhsT=w1d_ap(hk, dk),
                    rhs=dec_ap(dk),
                    start=(dk == 0), stop=(dk == DK - 1),
                )
            nc.scalar.activation(dech_p[q][:, r * BU:(r + 1) * BU], pd[:],
                                 mybir.ActivationFunctionType.Identity)
            # overlay: as soon as pair q is complete, pre-build (+tanh) that
            # pair of the first OVERLAY chunks
            if r == QHK - 1:
                for i in range(OVERLAY):
                    bo, to, tc = chunks[i]
                    build_pair(ov_st[i], bo, to, tc, q, fuse_tanh=True)


        # ---- phase 2 ----
        for ci, (b, t0c, tch) in enumerate(chunks):
            rows_c = tch * U
            if ci < OVERLAY:
                s_t = ov_st[ci]          # built during phase 1
            else:
                s_t = spool.tile([128, HK * CHROWS], BF16, tag="s")
                # DVE broadcast-add build (fp32 -> bf16), one per k-pair,
                # then one in-place tanh over all k
                for q in range(HK // QHK):
                    build_pair(s_t, b, t0c, tch, q, fuse_tanh=False)
                s_used = s_t[:].rearrange(
                    "p (k c) -> p k c", k=HK)[:, :, :rows_c]
                nc.scalar.activation(s_used, s_used,
                                     mybir.ActivationFunctionType.Tanh)
            row0 = b * (TLOC * U) + t0c * U
            # swapped matmul: W2 blocks stationary, s moving -> psum holds
            # out^T [o-chunk, rows]; b2 folds into the psum->sbuf copy as a
            # per-partition bias; all 4 oc slices land in one staging tile
            # so the chunk's output is a single DMA.
            ot = opool.tile([128, 4 * CHROWS], BF16, tag="ot")
            for oc in range(O // 128):
                ps = psB.tile([128, 512], F32, tag="psB")
                for k in range(HK):
                    nc.tensor.matmul(
                        ps[:, :rows_c],
                        lhsT=w2_ap(k, oc),
                        rhs=s_t[:, k * CHROWS: k * CHROWS + rows_c],
                        start=(k == 0), stop=(k == HK - 1),
                    )
                oslice = ot[:, oc * CHROWS: oc * CHROWS + rows_c]
                if oc < 2:
                    nc.scalar.activation(
                        oslice, ps[:, :rows_c],
                        mybir.ActivationFunctionType.Identity,
                        bias=b2c_s[:, oc:oc + 1])
                else:
                    nc.vector.tensor_scalar_add(
                        oslice, ps[:, :rows_c], b2c_s[:, oc:oc + 1])
            dst = out[:, row0:row0 + rows_c].rearrange(
                "(oc p) r -> p oc r", p=128)
            src = ot[:].rearrange(
                "p (oc c) -> p oc c", oc=4)[:, :, :rows_c]
            ring = nc.sync if ci % 2 == 0 else nc.scalar
            ring.dma_start(dst, src)
    nc.compile()
    return nc


def kernel(enc_state, dec_state, W1, b1, W2, b2, _trace=False):
    enc_state = np.ascontiguousarray(enc_state, dtype=np.float32)
    dec_state = np.ascontiguousarray(dec_state, dtype=np.float32)
    W1 = np.asarray(W1, dtype=np.float32)
    b1 = np.asarray(b1, dtype=np.float32)
    W2 = np.asarray(W2, dtype=np.float32)
    b2 = np.asarray(b2, dtype=np.float32)

    if "nc" not in _CACHE:
        _CACHE["nc"] = _build()
    nc = _CACHE["nc"]

    def chunk128(a):
        # [n*128, w] -> [128, n*w]: partition p holds row k*128+p of chunk k
        n = a.shape[0] // 128
        return np.ascontiguousarray(
            a.reshape(n, 128, a.shape[1]).transpose(1, 0, 2).reshape(128, -1))

    def hk_major(w):
        # chunk128 of [D, H] -> [128, dk, hk, 128]; reorder to [128, hk, dk, 128]
        c = chunk128(w)  # [128, DK*H]
        return np.ascontiguousarray(
            c.reshape(128, DK, HK, 128).transpose(0, 2, 1, 3).reshape(128, -1))

    def oc_major(w2c):
        # chunk128 of [H, O] = [128, hk, oc, 128]; reorder to [128, oc, hk, 128]
        return np.ascontiguousarray(
            w2c.reshape(128, HK, O // 128, 128).transpose(0, 2, 1, 3)
            .reshape(128, -1))

    decT = chunk128(dec_state.reshape(B * U, D).T).astype(NPBF16)
    w1eT = hk_major(W1[:, :D].T.astype(NPBF16))   # [128, hk, dk, 128]
    w1dT = hk_major(W1[:, D:].T.astype(NPBF16))
    # piece q = [w1e hk(2q..) | w1d hk(2q..)]; biases + dec dk0-1 ride in
    # piece 0, dec dk2-3 in piece 1
    QW = 2 * D
    pieces = [np.concatenate([w1eT[:, q * QW:(q + 1) * QW],
                              w1dT[:, q * QW:(q + 1) * QW]], axis=1)
              for q in range(4)]
    b1r = np.ascontiguousarray(b1.reshape(HK, 128).T).astype(NPBF16)    # [128, HK]
    b2cm = np.ascontiguousarray(b2.reshape(O // 128, 128).T).astype(NPBF16)
    BU2 = 2 * B * U
    w1qT = np.ascontiguousarray(np.concatenate(
        [pieces[0], b1r, b2cm, decT[:, :BU2], pieces[1], decT[:, BU2:],
         pieces[2], pieces[3]], axis=1))
    w2T = oc_major(chunk128(W2.T.astype(NPBF16)))

    in_maps = []
    for c in range(NCORES):
        enc_c = enc_state[:, c * TLOC:(c + 1) * TLOC, :].reshape(PAIRS, D)
        encT_c = chunk128(enc_c.T).astype(NPBF16)                       # [128, 4*100]
        in_maps.append({
            "encT": encT_c, "w1qT": w1qT, "w2T": w2T,
        })

    res = run_bass_kernel_spmd(nc, in_maps, list(range(NCORES)), trace=_trace)
    out = np.empty((B, T, U, O), dtype=np.float32)
    for c in range(NCORES):
        # device output is transposed: [O, ROWS]
        out[:, c * TLOC:(c + 1) * TLOC] = (
            res.results[c]["out"].astype(np.float32).T.reshape(B, TLOC, U, O))
    if _trace:
        kernel.last_results = res
    return out


# revision 47
# speedup vs baseline: 1.0060x; 1.0060x over previous
"""RNN-T joint network kernel for 8 Trainium2 NeuronCores.

out[b,t,u,:] = W2 @ tanh(W1e @ enc[b,t] + W1d @ dec[b,u] + b1) + b2

Shapes: B=4, T=200, U=100, D=512, H=1024, O=512.
Sharding: T split 8 ways (25 t's per core); dec + weights replicated.

All matmul inputs are bf16 (converted on host): halves input DMA, enables
FWL fast weight loads, and removes the fp32r cast pass.  Output is written
bf16 and upcast on host (norm rel err ~3.5e-3, well under the 2e-2 gate).
Phase-2 matmul streaming (32 MMs x rows per 500-row chunk = 133us/core at
1 col/cycle) is the hard floor; everything else is scheduled to hide under
the bandwidth-capped input load (~3.5MB/core, ~8.5-18us).

Per-core device program:
  Warmup: ~40 dummy matmuls keep the PE busy from the end of the engine
          preamble so the HAM clock-gate is at 8/8 (2.4 GHz) for phase 1.
  Input:  8 DMAs of ~0.5-0.7MB on the two HWDGE rings in strict need-order:
          encT, then w1 quarter-PAIRS (piece q = [w1e hk2q,2q+1 | w1d ...],
          with the bf16 biases and decT halves riding inside pieces 0/1 --
          tiny standalone DMAs poison a queue), then the two oc-major w2
          halves last.
  Phase 1: per hk, enc then dec matmuls (4 dk accumulating each), paced by
          the arriving w1 pieces; PSUM evacs (+b1) run on ACT into k-PAIR
          ench/dech tiles so the DVE FIFO holds only builds.
  Overlay: the first 3 chunks' s-tiles are built pair-by-pair inside the
          phase-1 loop (DVE broadcast-add + ACT tanh per pair), so when the
          first w2 half lands the PE goes straight to dense phase-2 flow.
  Phase 2: per chunk (up to 5 t's = 500 rows): DVE builds s[k-pair] =
          dec_hT (+) enc_hT bcast (stride-0 APs, fp32 in -> bf16 out), one
          in-place tanh over all k (ACT), then 4 oc x 8 k accumulating bf16
          matmuls against stationary W2 blocks -> psum out^T [128, rows];
          +b2 on the psum->sbuf copy (oc0,1 ACT / oc2,3 DVE) into one
          [128, 4*rows] staging tile; ONE output DMA per chunk, rings
          alternating.  Lead-in chunks are [2,3] t's, drain chunks [4,1].
"""

from contextlib import ExitStack

import numpy as np
import ml_dtypes

import concourse.bacc as bacc
import concourse.bass as bass
import concourse.mybir as mybir
import concourse.tile as tile
from concourse.bass_utils import run_bass_kernel_spmd

F32 = mybir.dt.float32
BF16 = mybir.dt.bfloat16
NPBF16 = ml_dtypes.bfloat16

B, T, U, D, H, O = 4, 200, 100, 512, 1024, 512
NCORES = 8
TLOC = T // NCORES            # 25 t's per core
PAIRS = B * TLOC              # 100 (b,t) pairs per core
TCH = 5                       # t's per inner chunk
CHROWS = TCH * U              # 500 rows per chunk
NCH = TLOC // TCH             # 5 chunks per b
ROWS = PAIRS * U              # 10000 output rows per core
DK = D // 128                 # 4 contraction chunks for phase 1
HK = H // 128                 # 8 h chunks

_CACHE = {}


def _build():
    nc = bacc.Bacc("TRN2", target_bir_lowering=False, debug=False,
                   num_devices=NCORES)
    # inputs arrive pre-interleaved in SBUF layout: [128, nchunk*width],
    # partition p holding chunk k's row (k*128+p) at cols [k*width, ...)
    encT = nc.dram_tensor("encT", [128, DK * PAIRS], BF16, kind="ExternalInput")
    # w1 is delivered as quarter-PAIRS: piece q = [w1e hk(2q,2q+1) | w1d
    # hk(2q,2q+1)], each 0.5MB -- big enough for good DMA throughput, small
    # enough that phase-1 paces along with the stream.  The bf16 biases ride
    # at the tail of piece 0: standalone 32B-descriptor DMAs poison a HWDGE
    # queue for microseconds.
    BIASC = HK + O // 128
    # decT halves ride inside pieces 0 and 1 so the dec data arrives with
    # its weights (no separate decT transfer to stall on)
    W1QC = 2 * HK * D + BIASC + DK * B * U
    w1qT = nc.dram_tensor("w1qT", [128, W1QC], BF16, kind="ExternalInput")
    # w2 layout is oc-major: [128, oc, hk, 128] -> the first matmul groups
    # (oc=0,1) only need the first half
    w2T = nc.dram_tensor("w2T", [128, HK * O], BF16, kind="ExternalInput")
    out = nc.dram_tensor("out", [O, ROWS], BF16, kind="ExternalOutput")

    BU = B * U
    QHK = 2                    # hk per w1 quarter
    with tile.TileContext(nc) as tc, ExitStack() as ctx:
        consts = ctx.enter_context(tc.tile_pool(name="consts", bufs=1))
        spool = ctx.enter_context(tc.tile_pool(name="spool", bufs=4))
        opool = ctx.enter_context(tc.tile_pool(name="opool", bufs=4))
        psB = ctx.enter_context(tc.tile_pool(name="psB", bufs=8, space="PSUM"))

        # ---- PE warmup: dummy matmuls so HAM un-throttles before phase 1 ----
        warm = consts.tile([128, 128], BF16, name="warm")
        nc.vector.memset(warm[:], 0.0)
        for _ in range(40):
            pw = psB.tile([128, 512], F32, tag="psB", name="pw")
            nc.tensor.matmul(pw[:, :128], lhsT=warm[:], rhs=warm[:],
                             start=True, stop=True)

        # ---- load inputs: both HWDGE rings, ~0.5MB pieces in strict
        # need-order, w2 halves last.  All 8 cores pull their copies
        # concurrently so the load is bandwidth-capped; phase 1 paces
        # along with the arriving quarter-pairs.
        QW = 2 * QHK * D
        L0 = QW + BIASC + 2 * BU      # piece0 + biases + dec dk0-1
        L1 = QW + 2 * BU              # piece1 + dec dk2-3
        w1q_s = [consts.tile([128, [L0, L1, QW, QW][i]], BF16,
                             name=f"w1q{i}") for i in range(4)]
        w2_s = [consts.tile([128, 2 * H], BF16, name=f"w2{i}")
                for i in range(2)]
        encT_s = consts.tile([128, DK * PAIRS], BF16)
        nc.sync.dma_start(encT_s[:], encT[:])
        nc.scalar.dma_start(w1q_s[0][:], w1qT[:, :L0])
        nc.sync.dma_start(w1q_s[1][:], w1qT[:, L0:L0 + L1])
        nc.scalar.dma_start(w1q_s[2][:], w1qT[:, L0 + L1:][:, :QW])
        nc.sync.dma_start(w1q_s[3][:], w1qT[:, L0 + L1 + QW:][:, :QW])
        nc.scalar.dma_start(w2_s[0][:], w2T[:, :2 * H])
        nc.sync.dma_start(w2_s[1][:], w2T[:, 2 * H:])
        # biases ride bf16 in the w1q0 piece; cast to fp32 once (DVE wants
        # fp32 scalars)
        b12f = consts.tile([128, BIASC], F32)
        nc.vector.tensor_copy(b12f[:], w1q_s[0][:, QW:QW + BIASC])
        b1_s = b12f[:, :HK]
        b2c_s = b12f[:, HK:]

        def dec_ap(dk):
            if dk < 2:
                return w1q_s[0][:, QW + BIASC + dk * BU:][:, :BU]
            return w1q_s[1][:, QW + (dk - 2) * BU:][:, :BU]

        def w1e_ap(hk, dk):
            return w1q_s[hk // QHK][:, (hk % QHK) * D + dk * 128:][:, :128]

        def w1d_ap(hk, dk):
            return w1q_s[hk // QHK][:, QHK * D + (hk % QHK) * D + dk * 128:][:, :128]

        def w2_ap(k, oc):
            return w2_s[oc // 2][:, (oc % 2) * H + k * 128:][:, :128]

        # ---- phase 1: enc_hT (+b1) and dec_hT, interleaved per hk ----
        # k-PAIR tiles (matching the w1 quarter DMAs): fine-grained enough
        # that phase-2 builds start as each pair is ready, and wide enough
        # that each build covers two k's in one DVE instruction.
        # evacs live on ACT so the DVE FIFO holds only builds.
        NP_ = HK // QHK
        ench_p = [consts.tile([128, QHK * PAIRS], F32, name=f"ench{q}")
                  for q in range(NP_)]
        dech_p = [consts.tile([128, QHK * BU], F32, name=f"dech{q}")
                  for q in range(NP_)]

        # phase-2 chunk table; the first OVERLAY chunks' builds+tanh are
        # emitted inside the phase-1 loop (pair-by-pair, as each ench/dech
        # pair lands) so all elementwise work for them is finished by the
        # time the w2 halves arrive and the PE can go dense immediately
        chunks = []
        for b in range(B):
            if b == 0:
                sizes = [2, 3] + [TCH] * 4
            elif b == B - 1:
                sizes = [TCH] * 4 + [4, 1]
            else:
                sizes = [TCH] * NCH
            t0c = 0
            for tch in sizes:
                chunks.append((b, t0c, tch))
                t0c += tch
        OVERLAY = 3
        ov_st = [spool.tile([128, HK * CHROWS], BF16, tag="s", name=f"sov{i}")
                 for i in range(OVERLAY)]

        def build_pair(s_t, b, t0c, tch, q, fuse_tanh):
            rows_c = tch * U
            c0 = b * TLOC + t0c
            in0 = dech_p[q][:].rearrange("p (k u) -> p k u", k=QHK)
            in0 = in0[:, :, b * U:(b + 1) * U].rearrange(
                "p k (a u) -> p k a u", a=1)
            in1 = ench_p[q][:].rearrange("p (k c) -> p k c", k=QHK)
            in1 = in1[:, :, c0:c0 + tch].rearrange(
                "p k (t a) -> p k t a", a=1)
            bc0, bc1 = bass.broadcast_tensor_aps(in0, in1)
            outap = s_t[:, q * QHK * CHROWS:(q + 1) * QHK * CHROWS]
            outap = outap.rearrange("p (k c) -> p k c", k=QHK)
            outap = outap[:, :, :rows_c].rearrange(
                "p k (t u) -> p k t u", t=tch)
            nc.vector.tensor_tensor(outap, bc0, bc1, mybir.AluOpType.add)
            if fuse_tanh:
                su = s_t[:, q * QHK * CHROWS:(q + 1) * QHK * CHROWS]
                su = su.rearrange("p (k c) -> p k c", k=QHK)[:, :, :rows_c]
                nc.scalar.activation(su, su,
                                     mybir.ActivationFunctionType.Tanh)

        for hk in range(HK):
            q, r = hk // QHK, hk % QHK
            pe = psB.tile([128, 512], F32, tag="psB", name="pe")
            pe = pe[:, :PAIRS]
            for dk in range(DK):
                nc.tensor.matmul(
                    pe[:],
                    lhsT=w1e_ap(hk, dk),
                    rhs=encT_s[:, dk * PAIRS:(dk + 1) * PAIRS],
                    start=(dk == 0), stop=(dk == DK - 1),
                )
            nc.scalar.activation(ench_p[q][:, r * PAIRS:(r + 1) * PAIRS], pe[:],
                                 mybir.ActivationFunctionType.Identity,
                                 bias=b1_s[:, hk:hk + 1])
            pd = psB.tile([128, 512], F32, tag="psB", name="pd")
            pd = pd[:, :BU]
            for dk in range(DK):
                nc.tensor.matmul(
                    pd[:],
                    lhsT=w1d_ap(hk, dk),
                    rhs=dec_ap(dk),
                    start=(dk == 0), stop=(dk == DK - 1),
                )
            nc.scalar.activation(dech_p[q][:, r * BU:(r + 1) * BU], pd[:],
                                 mybir.ActivationFunctionType.Identity)
            # overlay: as soon as pair q is complete, pre-build (+tanh) that
            # pair of the first OVERLAY chunks
            if r == QHK - 1:
                for i in range(OVERLAY):
                    bo, to, tc = chunks[i]
                    build_pair(ov_st[i], bo, to, tc, q, fuse_tanh=True)


        # ---- phase 2 ----
        # staging tiles + group emitter; overlay chunks split their matmul
        # groups: oc0,1 (first w2 half) for all three first, then oc2,3 --
        # ~6us of w2a-only PE work bridges until the second w2 half lands
        ov_ot = [opool.tile([128, 4 * CHROWS], BF16, tag="ot", name=f"ovot{i}")
                 for i in range(OVERLAY)]

        def mm_group(s_t, ot, tch, oc):
            rows_c = tch * U
            ps = psB.tile([128, 512], F32, tag="psB", name="ps")
            for k in range(HK):
                nc.tensor.matmul(
                    ps[:, :rows_c],
                    lhsT=w2_ap(k, oc),
                    rhs=s_t[:, k * CHROWS: k * CHROWS + rows_c],
                    start=(k == 0), stop=(k == HK - 1),
                )
            oslice = ot[:, oc * CHROWS: oc * CHROWS + rows_c]
            if oc < 2:
                nc.scalar.activation(
                    oslice, ps[:, :rows_c],
                    mybir.ActivationFunctionType.Identity,
                    bias=b2c_s[:, oc:oc + 1])
            else:
                nc.vector.tensor_scalar_add(
                    oslice, ps[:, :rows_c], b2c_s[:, oc:oc + 1])

        def chunk_dma(ot, ci, b, t0c, tch):
            rows_c = tch * U
            row0 = b * (TLOC * U) + t0c * U
            dst = out[:, row0:row0 + rows_c].rearrange(
                "(oc p) r -> p oc r", p=128)
            srcap = ot[:].rearrange(
                "p (oc c) -> p oc c", oc=4)[:, :, :rows_c]
            ring = nc.sync if ci % 2 == 0 else nc.scalar
            ring.dma_start(dst, srcap)

        for ci in range(OVERLAY):
            b, t0c, tch = chunks[ci]
            for oc in (0, 1):
                mm_group(ov_st[ci], ov_ot[ci], tch, oc)
        for ci in range(OVERLAY):
            b, t0c, tch = chunks[ci]
            for oc in (2, 3):
                mm_group(ov_st[ci], ov_ot[ci], tch, oc)
            chunk_dma(ov_ot[ci], ci, b, t0c, tch)
        for ci, (b, t0c, tch) in enumerate(chunks):
            if ci < OVERLAY:
                continue
            rows_c = tch * U
            s_t = spool.tile([128, HK * CHROWS], BF16, tag="s")
            for q in range(HK // QHK):
                build_pair(s_t, b, t0c, tch, q, fuse_tanh=False)
            s_used = s_t[:].rearrange(
                "p (k c) -> p k c", k=HK)[:, :, :rows_c]
            nc.scalar.activation(s_used, s_used,
                                 mybir.ActivationFunctionType.Tanh)
            ot = opool.tile([128, 4 * CHROWS], BF16, tag="ot")
            for oc in range(O // 128):
                mm_group(s_t, ot, tch, oc)
            chunk_dma(ot, ci, b, t0c, tch)
    nc.compile()
    return nc


def kernel(enc_state, dec_state, W1, b1, W2, b2, _trace=False):
    enc_state = np.ascontiguousarray(enc_state, dtype=np.float32)
    dec_state = np.ascontiguousarray(dec_state, dtype=np.float32)
    W1 = np.asarray(W1, dtype=np.float32)
    b1 = np.asarray(b1, dtype=np.float32)
    W2 = np.asarray(W2, dtype=np.float32)
    b2 = np.asarray(b2, dtype=np.float32)

    if "nc" not in _CACHE:
        _CACHE["nc"] = _build()
    nc = _CACHE["nc"]

    def chunk128(a):
        # [n*128, w] -> [128, n*w]: partition p holds row k*128+p of chunk k
        n = a.shape[0] // 128
        return np.ascontiguousarray(
            a.reshape(n, 128, a.shape[1]).transpose(1, 0, 2).reshape(128, -1))

    def hk_major(w):
        # chunk128 of [D, H] -> [128, dk, hk, 128]; reorder to [128, hk, dk, 128]
        c = chunk128(w)  # [128, DK*H]
        return np.ascontiguousarray(
            c.reshape(128, DK, HK, 128).transpose(0, 2, 1, 3).reshape(128, -1))

    def oc_major(w2c):
        # chunk128 of [H, O] = [128, hk, oc, 128]; reorder to [128, oc, hk, 128]
        return np.ascontiguousarray(
            w2c.reshape(128, HK, O // 128, 128).transpose(0, 2, 1, 3)
            .reshape(128, -1))

    decT = chunk128(dec_state.reshape(B * U, D).T).astype(NPBF16)
    w1eT = hk_major(W1[:, :D].T.astype(NPBF16))   # [128, hk, dk, 128]
    w1dT = hk_major(W1[:, D:].T.astype(NPBF16))
    # piece q = [w1e hk(2q..) | w1d hk(2q..)]; biases + dec dk0-1 ride in
    # piece 0, dec dk2-3 in piece 1
    QW = 2 * D
    pieces = [np.concatenate([w1eT[:, q * QW:(q + 1) * QW],
                              w1dT[:, q * QW:(q + 1) * QW]], axis=1)
              for q in range(4)]
    b1r = np.ascontiguousarray(b1.reshape(HK, 128).T).astype(NPBF16)    # [128, HK]
    b2cm = np.ascontiguousarray(b2.reshape(O // 128, 128).T).astype(NPBF16)
    BU2 = 2 * B * U
    w1qT = np.ascontiguousarray(np.concatenate(
        [pieces[0], b1r, b2cm, decT[:, :BU2], pieces[1], decT[:, BU2:],
         pieces[2], pieces[3]], axis=1))
    w2T = oc_major(chunk128(W2.T.astype(NPBF16)))

    in_maps = []
    for c in range(NCORES):
        enc_c = enc_state[:, c * TLOC:(c + 1) * TLOC, :].reshape(PAIRS, D)
        encT_c = chunk128(enc_c.T).astype(NPBF16)                       # [128, 4*100]
        in_maps.append({
            "encT": encT_c, "w1qT": w1qT, "w2T": w2T,
        })

    res = run_bass_kernel_spmd(nc, in_maps, list(range(NCORES)), trace=_trace)
    out = np.empty((B, T, U, O), dtype=np.float32)
    for c in range(NCORES):
        # device output is transposed: [O, ROWS]
        out[:, c * TLOC:(c + 1) * TLOC] = (
            res.results[c]["out"].astype(np.float32).T.reshape(B, TLOC, U, O))
    if _trace:
        kernel.last_results = res
    return out
